# revision 10
# baseline (speedup 1.0000x reference)
# Trainium2 Bass kernel for the NgramEnhancer pooling module.
# Self-contained: builds an SPMD Bass/Tile program for 8 NeuronCores,
# shards the word axis (1024 -> 8 x 128), runs on HW, gathers output.
import numpy as np
import ml_dtypes

DIMS = [32, 64, 128, 256, 512, 512, 512]
MAXC = 50
NW = 1024
NCORES = 8
W = NW // NCORES  # words per core = 128
DA = 100          # low-level attention dim
DN = 128          # high-level ngram space
DH = 256          # high-level attention dim
EPS = 1e-9
NORD = 7

GS = [MAXC - i for i in range(NORD)]            # g per order: 50..44
GPADS = [g + (g % 2) for g in GS]               # pad odd g to even
BWS = [512 // gp for gp in GPADS]               # words per psum slice

BF16 = ml_dtypes.bfloat16

_PROG = {}
LAST_RESULT = {}


def _slices(gp):
    bw = 8  # uniform slices: 16 x 8 words, N = 8*gp <= 400 fits one PSUM bank
    return [(w0, bw) for w0 in range(0, W, bw)]


def _build_program():
    import concourse.bacc as bacc
    import concourse.tile as tile
    import concourse.mybir as mybir

    dt = mybir.dt
    AF = mybir.ActivationFunctionType
    ALU = mybir.AluOpType

    nc = bacc.Bacc("TRN2", target_bir_lowering=False, debug=False, num_devices=NCORES)

    # ---- DRAM tensors (per-core shard) ----
    xs = [
        nc.dram_tensor(f"x{i}", [DIMS[i], W * GPADS[i]], dt.float32, kind="ExternalInput").ap()
        for i in range(NORD)
    ]
    cli_d = nc.dram_tensor("cli_all", [W, NORD], dt.float32, kind="ExternalInput").ap()
    jt_d = nc.dram_tensor("jt", [128, MAXC], dt.float32, kind="ExternalInput").ap()
    ident_d = nc.dram_tensor("ident", [128, 128], dt.float32, kind="ExternalInput").ap()

    NCH = [(c + 127) // 128 for c in DIMS]
    wl_d = [nc.dram_tensor(f"wl{i}", [128, NCH[i] * DA], dt.bfloat16, kind="ExternalInput").ap() for i in range(NORD)]
    pb_d = [nc.dram_tensor(f"pb{i}", [GS[i], DA], dt.bfloat16, kind="ExternalInput").ap() for i in range(NORD)]
    indic_d = [
        nc.dram_tensor(f"indic{i}", [GS[i], BWS[i] * GPADS[i]], dt.bfloat16, kind="ExternalInput").ap()
        for i in range(NORD)
    ]
    vlw_d = [nc.dram_tensor(f"vlw{i}", [DA, 32], dt.bfloat16, kind="ExternalInput").ap() for i in range(NORD)]
    wh_d = [nc.dram_tensor(f"wh{i}", [128, NCH[i] * DN], dt.bfloat16, kind="ExternalInput").ap() for i in range(NORD)]
    bh_d = [nc.dram_tensor(f"bh{i}", [1, DN], dt.bfloat16, kind="ExternalInput").ap() for i in range(NORD)]
    uw_d = nc.dram_tensor("uw", [DN, DH], dt.bfloat16, kind="ExternalInput").ap()
    ub_d = nc.dram_tensor("ub", [1, DH], dt.bfloat16, kind="ExternalInput").ap()
    vhw_d = nc.dram_tensor("vhw", [DN, 64], dt.bfloat16, kind="ExternalInput").ap()
    vhb_d = nc.dram_tensor("vhb", [1, 32], dt.bfloat16, kind="ExternalInput").ap()
    ones_d = nc.dram_tensor("ones_row", [1, W], dt.bfloat16, kind="ExternalInput").ap()

    CSUM = int(np.sum(DIMS))  # 2016
    out_d = nc.dram_tensor("out", [W, CSUM], dt.float32, kind="ExternalOutput").ap()
    u_scr = nc.dram_tensor("u_scratch", [NORD, W], dt.float32, kind="Internal").ap()

    COFF = np.concatenate([[0], np.cumsum(DIMS)]).astype(int)

    with tile.TileContext(nc) as tc:
        with (
            tc.tile_pool(name="const", bufs=1) as cpool,
            tc.tile_pool(name="small", bufs=2) as spool,
            tc.tile_pool(name="persist", bufs=1) as ppool,
            tc.tile_pool(name="th", bufs=4) as thpool,
        ):
            # ---- load constants ----
            def cload(dram_ap, shape, dtype, tag):
                t = cpool.tile(shape, dtype, tag=tag, name=tag)
                nc.sync.dma_start(out=t[:, :], in_=dram_ap)
                return t

            jt = cload(jt_d, [128, MAXC], dt.float32, "jt")
            cli = cload(cli_d, [W, NORD], dt.float32, "cli")
            ident = cload(ident_d, [128, 128], dt.float32, "ident")
            uw = cload(uw_d, [DN, DH], dt.bfloat16, "uw")
            ub = cload(ub_d, [1, DH], dt.bfloat16, "ub")
            vhw = cload(vhw_d, [DN, 64], dt.bfloat16, "vhw")
            vhb = cload(vhb_d, [1, 32], dt.bfloat16, "vhb")
            ones = cload(ones_d, [1, W], dt.bfloat16, "ones")
            wl = [cload(wl_d[i], [128, NCH[i] * DA], dt.bfloat16, f"wl{i}") for i in range(NORD)]
            pb = [cload(pb_d[i], [GS[i], DA], dt.bfloat16, f"pb{i}") for i in range(NORD)]
            vlw = [cload(vlw_d[i], [DA, 32], dt.bfloat16, f"vlw{i}") for i in range(NORD)]
            indic = [cload(indic_d[i], [GS[i], BWS[i] * GPADS[i]], dt.bfloat16, f"ind{i}") for i in range(NORD)]
            wh = [cload(wh_d[i], [128, NCH[i] * DN], dt.bfloat16, f"wh{i}") for i in range(NORD)]
            bh = [cload(bh_d[i], [1, DN], dt.bfloat16, f"bh{i}") for i in range(NORD)]

            # persistent pooled^T tiles (c_chunk, W)
            pooledT = {}
            pooledTb = {}
            for i in range(NORD):
                nch = (DIMS[i] + 127) // 128
                for k in range(nch):
                    ck = min(128, DIMS[i] - 128 * k)
                    pooledT[(i, k)] = ppool.tile([ck, W], dt.float32, tag=f"pt{i}_{k}", name=f"pt{i}_{k}")
                    pooledTb[(i, k)] = ppool.tile([ck, W], dt.bfloat16, tag=f"ptb{i}_{k}", name=f"ptb{i}_{k}")
            projb = [ppool.tile([DN, W], dt.bfloat16, tag=f"projb{i}", name=f"projb{i}") for i in range(NORD)]
            ah = ppool.tile([W, NORD], dt.float32, tag="ah")

            # ================= low-level attention per order =================
            with (
                tc.tile_pool(name="xp", bufs=5) as xpool,
                tc.tile_pool(name="erep", bufs=2) as epool,
                tc.tile_pool(name="yp", bufs=2) as ypool,
                tc.tile_pool(name="arow", bufs=1) as arpool,
                tc.tile_pool(name="ph", bufs=3, space="PSUM") as phpool,
                tc.tile_pool(name="psc", bufs=4, space="PSUM") as pscpool,
            ):
                for i in range(NORD):
                    g, gp, c = GS[i], GPADS[i], DIMS[i]
                    nch = (c + 127) // 128
                    sl = _slices(gp)

                    # -- load x chunks (fp32 HBM -> bf16 SBUF, (c_k, W*gp)) --
                    xt = []
                    for k in range(nch):
                        ck = min(128, c - 128 * k)
                        t = xpool.tile([ck, W * gp], dt.bfloat16, tag="x")
                        nc.gpsimd.dma_start(out=t[:, :], in_=xs[i][128 * k:128 * k + ck, :])
                        xt.append((t, ck))

                    # -- h matmuls + tanh + score matmuls --
                    nsl = len(sl)
                    nbank = (nsl + 3) // 4
                    psc = [pscpool.tile([128, 512], dt.float32, tag="psc", name=f"psc{i}_{bi}") for bi in range(nbank)]
                    sc_sl = [spool.tile([128, 512], dt.float32, tag=f"sc_sl{bi}", name=f"sc_sl{i}_{bi}") for bi in range(nbank)]

                    for t_i, (w0, nw) in enumerate(sl):
                        N = nw * gp
                        phh = phpool.tile([DA, N], dt.float32, tag="ph")
                        for k in range(nch):
                            xtile, ck = xt[k]
                            nc.tensor.matmul(
                                phh[:, :], wl[i][0:ck, k * DA:(k + 1) * DA],
                                xtile[:, w0 * gp:(w0 + nw) * gp],
                                start=(k == 0), stop=False,
                            )
                        nc.tensor.matmul(
                            phh[:, :], pb[i][:, :], indic[i][:, 0:N],
                            start=False, stop=True,
                        )
                        tht = thpool.tile([DA, N], dt.bfloat16, tag="th")
                        nc.scalar.activation(tht[:, :], phh[:, :], AF.Tanh)
                        b, p = t_i // 4, 32 * (t_i % 4)
                        nc.tensor.matmul(
                            psc[b][p:p + 32, 0:N], vlw[i][:, :], tht[:, :],
                            tile_position=(0, p),
                        )

                    # -- extract scores: strided 4-row ACT copies --
                    NS = 8 * gp
                    for b in range(nbank):
                        nc.scalar.activation(
                            sc_sl[b][:, 0:NS], psc[b][:, 0:NS], AF.Copy,
                        )

                    # -- reshape slice-rows -> (W, gp) (SBUF-SBUF, order match) --
                    sc_wg = spool.tile([W, gp], dt.float32, tag="sc_wg")
                    for t_i, (w0, nw) in enumerate(sl):
                        nc.sync.dma_start(
                            out=sc_wg[w0:w0 + nw, :],
                            in_=sc_sl[t_i // 4][32 * (t_i % 4):32 * (t_i % 4) + 1, 0:nw * gp],
                        )

                    # -- softmax over g (rows = words) --
                    negm = spool.tile([W, 1], dt.float32, tag="negm")
                    nc.vector.tensor_reduce(negm[:, :], sc_wg[:, 0:g], axis=mybir.AxisListType.X,
                                            op=ALU.max, negate=True)
                    e_t = spool.tile([W, gp], dt.float32, tag="e_t")
                    nc.scalar.activation(e_t[:, 0:g], sc_wg[:, 0:g], AF.Exp, bias=negm[:, :])
                    em = spool.tile([W, gp], dt.float32, tag="em")
                    ssum = spool.tile([W, 1], dt.float32, tag="ssum")
                    nc.vector.scalar_tensor_tensor(
                        em[:, 0:g], jt[:, 0:g], cli[:, i:i + 1], e_t[:, 0:g],
                        op0=ALU.is_lt, op1=ALU.mult, accum_out=ssum[:, :],
                    )
                    nc.vector.tensor_scalar_add(ssum[:, :], ssum[:, :], EPS)
                    rcp = spool.tile([W, 1], dt.float32, tag="rcp")
                    nc.vector.reciprocal(rcp[:, :], ssum[:, :])
                    a_bf = spool.tile([W, gp], dt.bfloat16, tag="a_bf")
                    nc.vector.tensor_scalar(a_bf[:, 0:g], em[:, 0:g], rcp[:, :], None, op0=ALU.mult)
                    if gp != g:
                        nc.vector.memset(a_bf[:, g:gp], 0.0)

                    # -- replicate a across 128 partitions --
                    a_row = arpool.tile([1, W * gp], dt.bfloat16, tag="a_row", name="a_row")
                    nc.sync.dma_start(out=a_row[:, :], in_=a_bf[:, :])
                    erep = epool.tile([128, W * gp], dt.bfloat16, tag="erep")
                    nc.gpsimd.partition_broadcast(erep[:, :], a_row[:, :])

                    # -- pooled^T = segmented sum of x * a --
                    for k in range(nch):
                        xtile, ck = xt[k]
                        y = ypool.tile([ck, W * gp], dt.bfloat16, tag="y")
                        nc.vector.tensor_tensor(y[:, :], xtile[:, :], erep[0:ck, :], op=ALU.mult)
                        y3 = y[:, :].rearrange("c (w g) -> c w g", g=gp)
                        if gp > 32:
                            r = gp - 32
                            nc.vector.tensor_tensor(y3[:, :, 0:r], y3[:, :, 0:r], y3[:, :, 32:gp], op=ALU.add)
                        for hw_ in (16, 8):
                            lim = min(2 * hw_, gp)
                            nc.vector.tensor_tensor(
                                y3[:, :, 0:lim - hw_], y3[:, :, 0:lim - hw_], y3[:, :, hw_:lim], op=ALU.add
                            )
                        nc.vector.tensor_reduce(
                            pooledT[(i, k)][:, :], y3[:, :, 0:8], axis=mybir.AxisListType.X, op=ALU.add
                        )
                        nc.vector.tensor_copy(pooledTb[(i, k)][:, :], pooledT[(i, k)][:, :])

            # ================= high-level attention =================
            with (
                tc.tile_pool(name="pproj", bufs=2, space="PSUM") as pjpool,
                tc.tile_pool(name="pz", bufs=2, space="PSUM") as pzpool,
                tc.tile_pool(name="pu", bufs=2, space="PSUM") as pupool,
                tc.tile_pool(name="ptr", bufs=2, space="PSUM") as ptpool,
            ):
                u_sb = [spool.tile([128, W], dt.float32, tag=f"u_sb{bi}", name=f"u_sb{bi}") for bi in range(2)]
                pu = [pupool.tile([128, 512], dt.float32, tag="pu", name=f"pu{bi}") for bi in range(2)]
                for i in range(NORD):
                    c = DIMS[i]
                    nch = (c + 127) // 128
                    pproj = pjpool.tile([DN, W], dt.float32, tag="pproj")
                    for k in range(nch):
                        ck = min(128, c - 128 * k)
                        nc.tensor.matmul(
                            pproj[:, :], wh[i][0:ck, k * DN:(k + 1) * DN], pooledTb[(i, k)][:, :],
                            start=(k == 0), stop=False,
                        )
                    nc.tensor.matmul(pproj[:, :], bh[i][:, :], ones[:, :], start=False, stop=True)
                    nc.scalar.activation(projb[i][:, :], pproj[:, :], AF.Copy)

                    b, p = i // 4, 32 * (i % 4)
                    for h2 in range(2):
                        pz = pzpool.tile([128, W], dt.float32, tag="pz")
                        nc.tensor.matmul(pz[:, :], uw[:, 128 * h2:128 * (h2 + 1)], projb[i][:, :],
                                         start=True, stop=False)
                        nc.tensor.matmul(pz[:, :], ub[:, 128 * h2:128 * (h2 + 1)], ones[:, :],
                                         start=False, stop=True)
                        tz = thpool.tile([128, W], dt.bfloat16, tag="tz")
                        nc.scalar.activation(tz[:, :], pz[:, :], AF.Tanh)
                        nc.tensor.matmul(pu[b][p:p + 32, 0:W], vhw[:, 32 * h2:32 * (h2 + 1)], tz[:, :],
                                         start=(h2 == 0), stop=False, tile_position=(0, p))
                    nc.tensor.matmul(pu[b][p:p + 32, 0:W], vhb[:, :], ones[:, :],
                                     start=False, stop=True, tile_position=(0, p))

                for b in range(2):
                    rows = min(4, NORD - 4 * b)
                    nc.scalar.activation(
                        u_sb[b][0:32 * rows, :], pu[b][0:32 * rows, 0:W], AF.Copy
                    )

                # transpose u (NORD, W) -> (W, NORD) via DRAM round-trip
                for i_o in range(NORD):
                    nc.sync.dma_start(
                        out=u_scr[i_o:i_o + 1, :],
                        in_=u_sb[i_o // 4][32 * (i_o % 4):32 * (i_o % 4) + 1, :],
                    )
                u_wg = spool.tile([W, NORD], dt.float32, tag="u_wg")
                nc.sync.dma_start(out=u_wg[:, :], in_=u_scr.rearrange("o w -> w o"))

                negmu = spool.tile([W, 1], dt.float32, tag="negmu")
                nc.vector.tensor_reduce(negmu[:, :], u_wg[:, :], axis=mybir.AxisListType.X,
                                        op=ALU.max, negate=True)
                eu = spool.tile([W, NORD], dt.float32, tag="eu")
                nc.scalar.activation(eu[:, :], u_wg[:, :], AF.Exp, bias=negmu[:, :])
                su = spool.tile([W, 1], dt.float32, tag="su")
                nc.vector.tensor_reduce(su[:, :], eu[:, :], axis=mybir.AxisListType.X, op=ALU.add)
                nc.vector.tensor_scalar_add(su[:, :], su[:, :], EPS)
                rcu = spool.tile([W, 1], dt.float32, tag="rcu")
                nc.vector.reciprocal(rcu[:, :], su[:, :])
                nc.vector.tensor_scalar(ah[:, :], eu[:, :], rcu[:, :], None, op0=ALU.mult)

                # ---- final scale + transpose + store ----
                for i in range(NORD):
                    c = DIMS[i]
                    nch = (c + 127) // 128
                    for k in range(nch):
                        ck = min(128, c - 128 * k)
                        pt = ptpool.tile([W, ck], dt.float32, tag="ptr")
                        nc.tensor.transpose(pt[:, :], pooledT[(i, k)][:, :], ident[0:ck, 0:ck])
                        ot = thpool.tile([W, ck], dt.float32, tag="ot")
                        nc.vector.tensor_scalar(ot[:, :], pt[:, :], ah[:, i:i + 1], None, op0=ALU.mult)
                        nc.sync.dma_start(
                            out=out_d[:, int(COFF[i]) + 128 * k: int(COFF[i]) + 128 * k + ck],
                            in_=ot[:, :],
                        )

    nc.compile()
    return nc


def _host_prep(inputs):
    def npf(a):
        return np.asarray(a, dtype=np.float32)

    Wl = [npf(a) for a in inputs["Wl"]]
    bl = [npf(a) for a in inputs["bl"]]
    vlw_l = [npf(a) for a in inputs["vlw"]]
    pos = [npf(a) for a in inputs["pos"]]
    Wh = [npf(a) for a in inputs["Wh"]]
    bh_l = [npf(a) for a in inputs["bh"]]
    Uw = npf(inputs["Uw"])
    Ub = npf(inputs["Ub"])
    vhw = npf(inputs["vhw"])
    vhb = npf(inputs["vhb"])

    cl = np.asarray(inputs["char_lengths"]).reshape(-1).astype(np.float32)

    # pb = pos[1:g+1] @ Wl + bl  (g, DA). vlb cancels in the softmax exactly.
    pb = [(pos[i][1:GS[i] + 1] @ Wl[i] + bl[i]).astype(np.float32) for i in range(NORD)]

    consts = {}
    consts["jt"] = np.broadcast_to(np.arange(MAXC, dtype=np.float32), (128, MAXC)).copy()
    consts["ident"] = np.eye(128, dtype=np.float32)
    for i in range(NORD):
        nch = (DIMS[i] + 127) // 128
        wlp = np.zeros((128, nch * DA), np.float32)
        for k in range(nch):
            ck = min(128, DIMS[i] - 128 * k)
            wlp[:ck, k * DA:(k + 1) * DA] = Wl[i][128 * k:128 * k + ck]
        consts[f"wl{i}"] = wlp.astype(BF16)
        consts[f"pb{i}"] = pb[i].astype(BF16)
        ind = np.zeros((GS[i], BWS[i] * GPADS[i]), np.float32)
        for w in range(BWS[i]):
            ind[:, w * GPADS[i]:w * GPADS[i] + GS[i]] = np.eye(GS[i], dtype=np.float32)
        consts[f"indic{i}"] = ind.astype(BF16)
        whp = np.zeros((128, nch * DN), np.float32)
        for k in range(nch):
            ck = min(128, DIMS[i] - 128 * k)
            whp[:ck, k * DN:(k + 1) * DN] = Wh[i][128 * k:128 * k + ck]
        consts[f"wh{i}"] = whp.astype(BF16)
        consts[f"bh{i}"] = bh_l[i].reshape(1, DN).astype(BF16)
        consts[f"vlw{i}"] = np.tile(vlw_l[i].reshape(DA, 1), (1, 32)).astype(BF16)
    consts["uw"] = Uw.astype(BF16)
    consts["ub"] = Ub.reshape(1, DH).astype(BF16)
    consts["vhw"] = np.concatenate([np.tile(vhw[:DN].reshape(DN, 1), (1, 32)), np.tile(vhw[DN:].reshape(DN, 1), (1, 32))], axis=1).astype(BF16)
    consts["vhb"] = np.full((1, 32), float(np.asarray(vhb)), np.float32).astype(BF16)
    consts["ones_row"] = np.ones((1, W), BF16)

    in_maps = []
    for m in range(NCORES):
        w0 = m * W
        im = dict(consts)
        for i in range(NORD):
            xi = np.asarray(inputs[f"x{i}"], dtype=np.float32)[w0:w0 + W]  # (W, c, g)
            g, gp, c = GS[i], GPADS[i], DIMS[i]
            xr = np.zeros((c, W, gp), np.float32)
            xr[:, :, 0:g] = xi.transpose(1, 0, 2)
            im[f"x{i}"] = xr.reshape(c, W * gp)
        cli = np.empty((W, NORD), np.float32)
        for i in range(NORD):
            cli[:, i] = cl[w0:w0 + W] - i
        im["cli_all"] = cli
        in_maps.append(im)
    return in_maps


def _get_program():
    if "nc" not in _PROG:
        _PROG["nc"] = _build_program()
    return _PROG["nc"]


def kernel(**inputs):
    from concourse import bass_utils

    nc = _get_program()
    in_maps = _host_prep(inputs)
    res = bass_utils.run_bass_kernel_spmd(nc, in_maps, core_ids=list(range(NCORES)))
    LAST_RESULT["res"] = res
    out = np.concatenate([res.results[m]["out"] for m in range(NCORES)], axis=0)
    return out.astype(np.float32)


# revision 11
# speedup vs baseline: 1.0373x; 1.0373x over previous
# Trainium2 Bass kernel for the NgramEnhancer pooling module.
# Self-contained: builds an SPMD Bass/Tile program for 8 NeuronCores,
# shards the word axis (1024 -> 8 x 128), runs on HW, gathers output.
import numpy as np
import ml_dtypes

DIMS = [32, 64, 128, 256, 512, 512, 512]
MAXC = 50
NW = 1024
NCORES = 8
W = NW // NCORES  # words per core = 128
DA = 100          # low-level attention dim
DN = 128          # high-level ngram space
DH = 256          # high-level attention dim
EPS = 1e-9
NORD = 7

GS = [MAXC - i for i in range(NORD)]            # g per order: 50..44
GPADS = [g + (g % 2) for g in GS]               # pad odd g to even
BWS = [512 // gp for gp in GPADS]               # words per psum slice

BF16 = ml_dtypes.bfloat16

_PROG = {}
LAST_RESULT = {}


def _slices(gp):
    bw = 8  # uniform slices: 16 x 8 words, N = 8*gp <= 400 fits one PSUM bank
    return [(w0, bw) for w0 in range(0, W, bw)]


def _build_program():
    import concourse.bacc as bacc
    import concourse.tile as tile
    import concourse.mybir as mybir

    dt = mybir.dt
    AF = mybir.ActivationFunctionType
    ALU = mybir.AluOpType

    nc = bacc.Bacc("TRN2", target_bir_lowering=False, debug=False, num_devices=NCORES)

    # ---- DRAM tensors (per-core shard) ----
    xs = [
        nc.dram_tensor(f"x{i}", [DIMS[i], W * GPADS[i]], dt.float32, kind="ExternalInput").ap()
        for i in range(NORD)
    ]
    cli_d = nc.dram_tensor("cli_all", [W, NORD], dt.float32, kind="ExternalInput").ap()
    jt_d = nc.dram_tensor("jt", [128, MAXC], dt.float32, kind="ExternalInput").ap()
    ident_d = nc.dram_tensor("ident", [128, 128], dt.float32, kind="ExternalInput").ap()

    NCH = [(c + 127) // 128 for c in DIMS]
    wl_d = [nc.dram_tensor(f"wl{i}", [128, NCH[i] * DA], dt.bfloat16, kind="ExternalInput").ap() for i in range(NORD)]
    pb_d = [nc.dram_tensor(f"pb{i}", [GS[i], DA], dt.bfloat16, kind="ExternalInput").ap() for i in range(NORD)]
    indic_d = [
        nc.dram_tensor(f"indic{i}", [GS[i], BWS[i] * GPADS[i]], dt.bfloat16, kind="ExternalInput").ap()
        for i in range(NORD)
    ]
    vlw_d = [nc.dram_tensor(f"vlw{i}", [DA, 32], dt.bfloat16, kind="ExternalInput").ap() for i in range(NORD)]
    wh_d = [nc.dram_tensor(f"wh{i}", [128, NCH[i] * DN], dt.bfloat16, kind="ExternalInput").ap() for i in range(NORD)]
    bh_d = [nc.dram_tensor(f"bh{i}", [1, DN], dt.bfloat16, kind="ExternalInput").ap() for i in range(NORD)]
    uw_d = nc.dram_tensor("uw", [DN, DH], dt.bfloat16, kind="ExternalInput").ap()
    ub_d = nc.dram_tensor("ub", [1, DH], dt.bfloat16, kind="ExternalInput").ap()
    vhw_d = nc.dram_tensor("vhw", [DN, 64], dt.bfloat16, kind="ExternalInput").ap()
    vhb_d = nc.dram_tensor("vhb", [1, 32], dt.bfloat16, kind="ExternalInput").ap()
    ones_d = nc.dram_tensor("ones_row", [1, W], dt.bfloat16, kind="ExternalInput").ap()

    CSUM = int(np.sum(DIMS))  # 2016
    out_d = nc.dram_tensor("out", [W, CSUM], dt.float32, kind="ExternalOutput").ap()
    u_scr = nc.dram_tensor("u_scratch", [NORD, W], dt.float32, kind="Internal").ap()

    COFF = np.concatenate([[0], np.cumsum(DIMS)]).astype(int)

    with tile.TileContext(nc) as tc:
        with (
            tc.tile_pool(name="const", bufs=1) as cpool,
            tc.tile_pool(name="small", bufs=2) as spool,
            tc.tile_pool(name="persist", bufs=1) as ppool,
            tc.tile_pool(name="th", bufs=4) as thpool,
        ):
            # ---- load constants ----
            def cload(dram_ap, shape, dtype, tag):
                t = cpool.tile(shape, dtype, tag=tag, name=tag)
                nc.sync.dma_start(out=t[:, :], in_=dram_ap)
                return t

            jt = cload(jt_d, [128, MAXC], dt.float32, "jt")
            cli = cload(cli_d, [W, NORD], dt.float32, "cli")
            ident = cload(ident_d, [128, 128], dt.float32, "ident")
            uw = cload(uw_d, [DN, DH], dt.bfloat16, "uw")
            ub = cload(ub_d, [1, DH], dt.bfloat16, "ub")
            vhw = cload(vhw_d, [DN, 64], dt.bfloat16, "vhw")
            vhb = cload(vhb_d, [1, 32], dt.bfloat16, "vhb")
            ones = cload(ones_d, [1, W], dt.bfloat16, "ones")
            wl = [cload(wl_d[i], [128, NCH[i] * DA], dt.bfloat16, f"wl{i}") for i in range(NORD)]
            pb = [cload(pb_d[i], [GS[i], DA], dt.bfloat16, f"pb{i}") for i in range(NORD)]
            vlw = [cload(vlw_d[i], [DA, 32], dt.bfloat16, f"vlw{i}") for i in range(NORD)]
            indic = [cload(indic_d[i], [GS[i], BWS[i] * GPADS[i]], dt.bfloat16, f"ind{i}") for i in range(NORD)]
            wh = [cload(wh_d[i], [128, NCH[i] * DN], dt.bfloat16, f"wh{i}") for i in range(NORD)]
            bh = [cload(bh_d[i], [1, DN], dt.bfloat16, f"bh{i}") for i in range(NORD)]

            # persistent pooled^T tiles (c_chunk, W)
            pooledT = {}
            pooledTb = {}
            for i in range(NORD):
                nch = (DIMS[i] + 127) // 128
                for k in range(nch):
                    ck = min(128, DIMS[i] - 128 * k)
                    pooledT[(i, k)] = ppool.tile([ck, W], dt.float32, tag=f"pt{i}_{k}", name=f"pt{i}_{k}")
                    pooledTb[(i, k)] = ppool.tile([ck, W], dt.bfloat16, tag=f"ptb{i}_{k}", name=f"ptb{i}_{k}")
            projb = [ppool.tile([DN, W], dt.bfloat16, tag=f"projb{i}", name=f"projb{i}") for i in range(NORD)]
            ah = ppool.tile([W, NORD], dt.float32, tag="ah")

            # ================= low-level attention per order =================
            with (
                tc.tile_pool(name="xp", bufs=7) as xpool,
                tc.tile_pool(name="erep", bufs=2) as epool,
                tc.tile_pool(name="arow", bufs=1) as arpool,
                tc.tile_pool(name="ph", bufs=3, space="PSUM") as phpool,
                tc.tile_pool(name="psc", bufs=4, space="PSUM") as pscpool,
            ):
                for i in range(NORD):
                    g, gp, c = GS[i], GPADS[i], DIMS[i]
                    nch = (c + 127) // 128
                    sl = _slices(gp)

                    # -- load x chunks (fp32 HBM -> bf16 SBUF, (c_k, W*gp)) --
                    xt = []
                    for k in range(nch):
                        ck = min(128, c - 128 * k)
                        t = xpool.tile([ck, W * gp], dt.bfloat16, tag="x")
                        hf = W * gp // 2
                        nc.gpsimd.dma_start(out=t[:, 0:hf], in_=xs[i][128 * k:128 * k + ck, 0:hf])
                        nc.gpsimd.dma_start(out=t[:, hf:], in_=xs[i][128 * k:128 * k + ck, hf:])
                        xt.append((t, ck))

                    # -- h matmuls + tanh + score matmuls --
                    nsl = len(sl)
                    nbank = (nsl + 3) // 4
                    psc = [pscpool.tile([128, 512], dt.float32, tag="psc", name=f"psc{i}_{bi}") for bi in range(nbank)]
                    sc_sl = [spool.tile([128, 512], dt.float32, tag=f"sc_sl{bi}", name=f"sc_sl{i}_{bi}") for bi in range(nbank)]

                    for t_i, (w0, nw) in enumerate(sl):
                        N = nw * gp
                        phh = phpool.tile([DA, N], dt.float32, tag="ph")
                        for k in range(nch):
                            xtile, ck = xt[k]
                            nc.tensor.matmul(
                                phh[:, :], wl[i][0:ck, k * DA:(k + 1) * DA],
                                xtile[:, w0 * gp:(w0 + nw) * gp],
                                start=(k == 0), stop=False,
                            )
                        nc.tensor.matmul(
                            phh[:, :], pb[i][:, :], indic[i][:, 0:N],
                            start=False, stop=True,
                        )
                        tht = thpool.tile([DA, N], dt.bfloat16, tag="th")
                        nc.scalar.activation(tht[:, :], phh[:, :], AF.Tanh)
                        b, p = t_i // 4, 32 * (t_i % 4)
                        nc.tensor.matmul(
                            psc[b][p:p + 32, 0:N], vlw[i][:, :], tht[:, :],
                            tile_position=(0, p),
                        )

                    # -- extract scores: strided 4-row ACT copies --
                    NS = 8 * gp
                    for b in range(nbank):
                        nc.scalar.activation(
                            sc_sl[b][:, 0:NS], psc[b][:, 0:NS], AF.Copy,
                        )

                    # -- reshape slice-rows -> (W, gp) (SBUF-SBUF, order match) --
                    sc_wg = spool.tile([W, gp], dt.float32, tag="sc_wg")
                    for t_i, (w0, nw) in enumerate(sl):
                        nc.sync.dma_start(
                            out=sc_wg[w0:w0 + nw, :],
                            in_=sc_sl[t_i // 4][32 * (t_i % 4):32 * (t_i % 4) + 1, 0:nw * gp],
                        )

                    # -- softmax over g (rows = words) --
                    negm = spool.tile([W, 1], dt.float32, tag="negm")
                    nc.vector.tensor_reduce(negm[:, :], sc_wg[:, 0:g], axis=mybir.AxisListType.X,
                                            op=ALU.max, negate=True)
                    e_t = spool.tile([W, gp], dt.float32, tag="e_t")
                    nc.scalar.activation(e_t[:, 0:g], sc_wg[:, 0:g], AF.Exp, bias=negm[:, :])
                    em = spool.tile([W, gp], dt.float32, tag="em")
                    ssum = spool.tile([W, 1], dt.float32, tag="ssum")
                    nc.vector.scalar_tensor_tensor(
                        em[:, 0:g], jt[:, 0:g], cli[:, i:i + 1], e_t[:, 0:g],
                        op0=ALU.is_lt, op1=ALU.mult, accum_out=ssum[:, :],
                    )
                    nc.vector.tensor_scalar_add(ssum[:, :], ssum[:, :], EPS)
                    rcp = spool.tile([W, 1], dt.float32, tag="rcp")
                    nc.vector.reciprocal(rcp[:, :], ssum[:, :])
                    a_bf = spool.tile([W, gp], dt.bfloat16, tag="a_bf")
                    nc.vector.tensor_scalar(a_bf[:, 0:g], em[:, 0:g], rcp[:, :], None, op0=ALU.mult)
                    if gp != g:
                        nc.vector.memset(a_bf[:, g:gp], 0.0)

                    # -- replicate a across 128 partitions --
                    a_row = arpool.tile([1, W * gp], dt.bfloat16, tag="a_row", name="a_row")
                    nc.sync.dma_start(out=a_row[:, :], in_=a_bf[:, :])
                    erep = epool.tile([128, W * gp], dt.bfloat16, tag="erep")
                    nc.gpsimd.partition_broadcast(erep[:, :], a_row[:, :])

                    # -- pooled^T = segmented sum of x * a --
                    for k in range(nch):
                        xtile, ck = xt[k]
                        nc.vector.tensor_tensor(xtile[:, :], xtile[:, :], erep[0:ck, :], op=ALU.mult)
                        y3 = xtile[:, :].rearrange("c (w g) -> c w g", g=gp)
                        if gp > 32:
                            r = gp - 32
                            nc.vector.tensor_tensor(y3[:, :, 0:r], y3[:, :, 0:r], y3[:, :, 32:gp], op=ALU.add)
                        for hw_ in (16, 8):
                            lim = min(2 * hw_, gp)
                            nc.vector.tensor_tensor(
                                y3[:, :, 0:lim - hw_], y3[:, :, 0:lim - hw_], y3[:, :, hw_:lim], op=ALU.add
                            )
                        nc.vector.tensor_reduce(
                            pooledT[(i, k)][:, :], y3[:, :, 0:8], axis=mybir.AxisListType.X, op=ALU.add
                        )
                        nc.vector.tensor_copy(pooledTb[(i, k)][:, :], pooledT[(i, k)][:, :])

            # ================= high-level attention =================
            with (
                tc.tile_pool(name="pproj", bufs=2, space="PSUM") as pjpool,
                tc.tile_pool(name="pz", bufs=2, space="PSUM") as pzpool,
                tc.tile_pool(name="pu", bufs=2, space="PSUM") as pupool,
                tc.tile_pool(name="ptr", bufs=2, space="PSUM") as ptpool,
            ):
                u_sb = [spool.tile([128, W], dt.float32, tag=f"u_sb{bi}", name=f"u_sb{bi}") for bi in range(2)]
                pu = [pupool.tile([128, 512], dt.float32, tag="pu", name=f"pu{bi}") for bi in range(2)]
                for i in range(NORD):
                    c = DIMS[i]
                    nch = (c + 127) // 128
                    pproj = pjpool.tile([DN, W], dt.float32, tag="pproj")
                    for k in range(nch):
                        ck = min(128, c - 128 * k)
                        nc.tensor.matmul(
                            pproj[:, :], wh[i][0:ck, k * DN:(k + 1) * DN], pooledTb[(i, k)][:, :],
                            start=(k == 0), stop=False,
                        )
                    nc.tensor.matmul(pproj[:, :], bh[i][:, :], ones[:, :], start=False, stop=True)
                    nc.scalar.activation(projb[i][:, :], pproj[:, :], AF.Copy)

                    b, p = i // 4, 32 * (i % 4)
                    for h2 in range(2):
                        pz = pzpool.tile([128, W], dt.float32, tag="pz")
                        nc.tensor.matmul(pz[:, :], uw[:, 128 * h2:128 * (h2 + 1)], projb[i][:, :],
                                         start=True, stop=False)
                        nc.tensor.matmul(pz[:, :], ub[:, 128 * h2:128 * (h2 + 1)], ones[:, :],
                                         start=False, stop=True)
                        tz = thpool.tile([128, W], dt.bfloat16, tag="tz")
                        nc.scalar.activation(tz[:, :], pz[:, :], AF.Tanh)
                        nc.tensor.matmul(pu[b][p:p + 32, 0:W], vhw[:, 32 * h2:32 * (h2 + 1)], tz[:, :],
                                         start=(h2 == 0), stop=False, tile_position=(0, p))
                    nc.tensor.matmul(pu[b][p:p + 32, 0:W], vhb[:, :], ones[:, :],
                                     start=False, stop=True, tile_position=(0, p))

                for b in range(2):
                    rows = min(4, NORD - 4 * b)
                    nc.scalar.activation(
                        u_sb[b][0:32 * rows, :], pu[b][0:32 * rows, 0:W], AF.Copy
                    )

                # transpose u (NORD, W) -> (W, NORD) via DRAM round-trip
                for i_o in range(NORD):
                    nc.sync.dma_start(
                        out=u_scr[i_o:i_o + 1, :],
                        in_=u_sb[i_o // 4][32 * (i_o % 4):32 * (i_o % 4) + 1, :],
                    )
                u_wg = spool.tile([W, NORD], dt.float32, tag="u_wg")
                nc.sync.dma_start(out=u_wg[:, :], in_=u_scr.rearrange("o w -> w o"))

                negmu = spool.tile([W, 1], dt.float32, tag="negmu")
                nc.vector.tensor_reduce(negmu[:, :], u_wg[:, :], axis=mybir.AxisListType.X,
                                        op=ALU.max, negate=True)
                eu = spool.tile([W, NORD], dt.float32, tag="eu")
                nc.scalar.activation(eu[:, :], u_wg[:, :], AF.Exp, bias=negmu[:, :])
                su = spool.tile([W, 1], dt.float32, tag="su")
                nc.vector.tensor_reduce(su[:, :], eu[:, :], axis=mybir.AxisListType.X, op=ALU.add)
                nc.vector.tensor_scalar_add(su[:, :], su[:, :], EPS)
                rcu = spool.tile([W, 1], dt.float32, tag="rcu")
                nc.vector.reciprocal(rcu[:, :], su[:, :])
                nc.vector.tensor_scalar(ah[:, :], eu[:, :], rcu[:, :], None, op0=ALU.mult)

                # ---- final scale + transpose + store ----
                for i in range(NORD):
                    c = DIMS[i]
                    nch = (c + 127) // 128
                    for k in range(nch):
                        ck = min(128, c - 128 * k)
                        pt = ptpool.tile([W, ck], dt.float32, tag="ptr")
                        nc.tensor.transpose(pt[:, :], pooledT[(i, k)][:, :], ident[0:ck, 0:ck])
                        ot = thpool.tile([W, ck], dt.float32, tag="ot")
                        nc.vector.tensor_scalar(ot[:, :], pt[:, :], ah[:, i:i + 1], None, op0=ALU.mult)
                        nc.sync.dma_start(
                            out=out_d[:, int(COFF[i]) + 128 * k: int(COFF[i]) + 128 * k + ck],
                            in_=ot[:, :],
                        )

    nc.compile()
    return nc


def _host_prep(inputs):
    def npf(a):
        return np.asarray(a, dtype=np.float32)

    Wl = [npf(a) for a in inputs["Wl"]]
    bl = [npf(a) for a in inputs["bl"]]
    vlw_l = [npf(a) for a in inputs["vlw"]]
    pos = [npf(a) for a in inputs["pos"]]
    Wh = [npf(a) for a in inputs["Wh"]]
    bh_l = [npf(a) for a in inputs["bh"]]
    Uw = npf(inputs["Uw"])
    Ub = npf(inputs["Ub"])
    vhw = npf(inputs["vhw"])
    vhb = npf(inputs["vhb"])

    cl = np.asarray(inputs["char_lengths"]).reshape(-1).astype(np.float32)

    # pb = pos[1:g+1] @ Wl + bl  (g, DA). vlb cancels in the softmax exactly.
    pb = [(pos[i][1:GS[i] + 1] @ Wl[i] + bl[i]).astype(np.float32) for i in range(NORD)]

    consts = {}
    consts["jt"] = np.broadcast_to(np.arange(MAXC, dtype=np.float32), (128, MAXC)).copy()
    consts["ident"] = np.eye(128, dtype=np.float32)
    for i in range(NORD):
        nch = (DIMS[i] + 127) // 128
        wlp = np.zeros((128, nch * DA), np.float32)
        for k in range(nch):
            ck = min(128, DIMS[i] - 128 * k)
            wlp[:ck, k * DA:(k + 1) * DA] = Wl[i][128 * k:128 * k + ck]
        consts[f"wl{i}"] = wlp.astype(BF16)
        consts[f"pb{i}"] = pb[i].astype(BF16)
        ind = np.zeros((GS[i], BWS[i] * GPADS[i]), np.float32)
        for w in range(BWS[i]):
            ind[:, w * GPADS[i]:w * GPADS[i] + GS[i]] = np.eye(GS[i], dtype=np.float32)
        consts[f"indic{i}"] = ind.astype(BF16)
        whp = np.zeros((128, nch * DN), np.float32)
        for k in range(nch):
            ck = min(128, DIMS[i] - 128 * k)
            whp[:ck, k * DN:(k + 1) * DN] = Wh[i][128 * k:128 * k + ck]
        consts[f"wh{i}"] = whp.astype(BF16)
        consts[f"bh{i}"] = bh_l[i].reshape(1, DN).astype(BF16)
        consts[f"vlw{i}"] = np.tile(vlw_l[i].reshape(DA, 1), (1, 32)).astype(BF16)
    consts["uw"] = Uw.astype(BF16)
    consts["ub"] = Ub.reshape(1, DH).astype(BF16)
    consts["vhw"] = np.concatenate([np.tile(vhw[:DN].reshape(DN, 1), (1, 32)), np.tile(vhw[DN:].reshape(DN, 1), (1, 32))], axis=1).astype(BF16)
    consts["vhb"] = np.full((1, 32), float(np.asarray(vhb)), np.float32).astype(BF16)
    consts["ones_row"] = np.ones((1, W), BF16)

    in_maps = []
    for m in range(NCORES):
        w0 = m * W
        im = dict(consts)
        for i in range(NORD):
            xi = np.asarray(inputs[f"x{i}"], dtype=np.float32)[w0:w0 + W]  # (W, c, g)
            g, gp, c = GS[i], GPADS[i], DIMS[i]
            xr = np.zeros((c, W, gp), np.float32)
            xr[:, :, 0:g] = xi.transpose(1, 0, 2)
            im[f"x{i}"] = xr.reshape(c, W * gp)
        cli = np.empty((W, NORD), np.float32)
        for i in range(NORD):
            cli[:, i] = cl[w0:w0 + W] - i
        im["cli_all"] = cli
        in_maps.append(im)
    return in_maps


def _get_program():
    if "nc" not in _PROG:
        _PROG["nc"] = _build_program()
    return _PROG["nc"]


def kernel(**inputs):
    from concourse import bass_utils

    nc = _get_program()
    in_maps = _host_prep(inputs)
    res = bass_utils.run_bass_kernel_spmd(nc, in_maps, core_ids=list(range(NCORES)))
    LAST_RESULT["res"] = res
    out = np.concatenate([res.results[m]["out"] for m in range(NCORES)], axis=0)
    return out.astype(np.float32)
